# revision 1
# baseline (speedup 1.0000x reference)
"""Causal self-attention (B=2, S=2048, E=1024, H=16) on 8 TRN2 NeuronCores.

Sharding: core c = 4*b + g handles batch b and head-group g (4 heads,
256 E-columns). Each core computes q/k/v projections for its head slice,
causal flash-style attention for its 4 heads, and a partial output
projection y_c = ctx_g @ Wo[rows_g].  Host sums the 4 partials per batch
and adds bo.

Device dataflow (per core), f32r on all matmul paths:
  xT [E,S] (host-pretransposed) -> qT/kT [2x128, S] (head-major: head h in
  tile h//2, partitions (h%2)*64..) and v1 [S, 4x(64+1)] (natural layout +
  ones column -> softmax denominator rides the attention matmul).
  Per q-chunk (512) x head-pair: one [128,1024] PSUM tile holds both
  heads' scoresT for a k-tile (K=64 matmuls at base partitions 0/64 run
  concurrently in separate PE row groups), one ACT exp covers both heads,
  causal masking multiplies a 0/1 triangle into the diagonal 128-block
  (gpsimd), ctxT[65,512] += v1-tile.T @ expT (K=128; row 64 = denominator).
  Normalization: DVE reciprocal of the PSUM denominator row -> gpsimd
  partition_broadcast -> DVE multiply. Output projection uses ctxT as lhsT.
  Causal trimming: for diagonal k-tile t' only q-columns >= 128*t' are
  computed (scores matmul, exp, ctx matmul all restricted).
  Emission interleaves projection chunks with attention q-chunks so ACT
  (exp) work overlaps projection-phase PE work.
"""

import os

import numpy as np

os.environ.setdefault("NEURON_RT_RESET_CORES", "1")

B, S, E, H, D = 2, 2048, 1024, 16, 64
NCORES = 8
EC = 256          # E-columns per core (4 heads x 64)
QC = 512          # q-chunk width
NQC = S // QC     # 4
NKT = S // 128    # 16 k-tiles
NE = E // 128     # 8 contraction chunks

_CACHE = {}


def _build_nc(cfg=None):
    cfg = cfg or {}
    MM_BUFS = cfg.get("mm", 2)
    CX_BUFS = cfg.get("cx", 3)
    PY_BUFS = cfg.get("py", 1)
    EXP_BUFS = cfg.get("exp", 4)
    CTX_BUFS = cfg.get("ctx", 4)
    import concourse.mybir as mybir
    import concourse.tile as tile
    import concourse.bass as bass
    from concourse import bacc

    F32 = mybir.dt.float32
    F32R = mybir.dt.float32r
    EXP = mybir.ActivationFunctionType.Exp

    nc = bacc.Bacc("TRN2", target_bir_lowering=False, debug=False)

    xT = nc.dram_tensor("xT", [E, S], F32R, kind="ExternalInput")
    wq = nc.dram_tensor("wq", [E, EC], F32R, kind="ExternalInput")
    wk = nc.dram_tensor("wk", [E, EC], F32R, kind="ExternalInput")
    wv = nc.dram_tensor("wv", [E, EC], F32R, kind="ExternalInput")
    wo = nc.dram_tensor("wo", [EC, E], F32R, kind="ExternalInput")
    bq = nc.dram_tensor("bq", [2, 128, 1], F32, kind="ExternalInput")
    bk = nc.dram_tensor("bk", [2, 128, 1], F32, kind="ExternalInput")
    bv = nc.dram_tensor("bv", [1, EC], F32, kind="ExternalInput")
    msk = nc.dram_tensor("msk", [128, 128], F32R, kind="ExternalInput")
    ones = nc.dram_tensor("ones", [1, 64], F32R, kind="ExternalInput")

    y = nc.dram_tensor("y", [S, E], F32, kind="ExternalOutput")

    with tile.TileContext(nc) as tc:
        with (
            tc.tile_pool(name="weights", bufs=1) as wpool,
            tc.tile_pool(name="xtp", bufs=1) as xtp,
            tc.tile_pool(name="qkv", bufs=1) as qkv,
            tc.tile_pool(name="expp", bufs=EXP_BUFS) as expp,
            tc.tile_pool(name="ctxn", bufs=CTX_BUFS) as ctxp,
            tc.tile_pool(name="odd", bufs=2) as oddp,
            tc.tile_pool(name="yp", bufs=4) as yp,
            tc.tile_pool(name="rows", bufs=3) as rows,
            tc.tile_pool(name="smalls", bufs=1) as smalls,
            tc.tile_pool(name="mm", bufs=MM_BUFS, space="PSUM") as mmp,
            tc.tile_pool(name="cx", bufs=CX_BUFS, space="PSUM") as cxp,
            tc.tile_pool(name="pyp", bufs=PY_BUFS, space="PSUM") as pyp,
        ):
            # ---- small constants (SWDGE/Pool queue; SP stays free) ----
            tbq = smalls.tile([128, 2], F32, tag="bq")
            tbk = smalls.tile([128, 2], F32, tag="bk")
            tbv = smalls.tile([128, EC], F32, tag="bv")
            tmsk = smalls.tile([128, 128], F32R, tag="msk")
            tones = smalls.tile([1, 64], F32R, tag="ones")

            for r in range(2):
                nc.gpsimd.dma_start(tbq[:, r:r + 1], bq[r])
                nc.gpsimd.dma_start(tbk[:, r:r + 1], bk[r])
            bvap = bv[0, :]
            bv_b = bass.AP(tensor=bvap.tensor, offset=bvap.offset,
                           ap=[[0, 128]] + list(bvap.ap))
            nc.gpsimd.dma_start(tbv[:], bv_b)
            nc.gpsimd.dma_start(tmsk[:], msk[:])
            nc.gpsimd.dma_start(tones[:], ones[:])

            # ---- bulk inputs: single DMA per weight tensor ----
            twq = wpool.tile([128, NE, EC], F32R, tag="wq")
            twk = wpool.tile([128, NE, EC], F32R, tag="wk")
            twv = wpool.tile([128, NE, EC], F32R, tag="wv")
            two = wpool.tile([128, 2, E], F32R, tag="wo")

            def chunked(dram, nch, width):
                # [nch*128, width] DRAM -> [128, nch, width] SBUF view
                a = dram[:]
                return bass.AP(tensor=a.tensor, offset=a.offset,
                               ap=[[width, 128], [128 * width, nch], [1, width]])

            txt = [xtp.tile([128, S], F32R, tag=f"xt{e}", name=f"xt{e}")
                   for e in range(NE)]
            if cfg.get("ord", "B") == "B":
                nsp = cfg.get("nsplit", 4)
                def ldx(e):
                    w = S // nsp
                    for i in range(nsp):
                        nc.sync.dma_start(
                            txt[e][:, i * w:(i + 1) * w],
                            xT[e * 128:(e + 1) * 128, i * w:(i + 1) * w])
                ldx(0)
                nc.sync.dma_start(twq[:], chunked(wq, NE, EC))
                nc.sync.dma_start(twk[:], chunked(wk, NE, EC))
                nc.sync.dma_start(twv[:], chunked(wv, NE, EC))
                for e in range(1, NE):
                    ldx(e)
                nc.sync.dma_start(two[:], chunked(wo, 2, E))
            else:
                nc.sync.dma_start(txt[0][:], xT[0:128, :])
                nc.sync.dma_start(twq[:], chunked(wq, NE, EC))
                nc.sync.dma_start(txt[1][:], xT[128:256, :])
                nc.sync.dma_start(twk[:], chunked(wk, NE, EC))
                nc.sync.dma_start(txt[2][:], xT[256:384, :])
                nc.sync.dma_start(twv[:], chunked(wv, NE, EC))
                for e in range(3, NE):
                    nc.sync.dma_start(txt[e][:], xT[e * 128:(e + 1) * 128, :])
                nc.sync.dma_start(two[:], chunked(wo, 2, E))

            # ---- persistent activation tiles ----
            tq = [qkv.tile([128, S], F32R, tag=f"q{r}", name=f"q{r}")
                  for r in range(2)]
            tk = [qkv.tile([128, S], F32R, tag=f"k{r}", name=f"k{r}")
                  for r in range(2)]
            # v1: [128, s-tile, head, 65]; col 64 of each head block = 1.0
            tv = qkv.tile([128, NKT, 4, 65], F32R, tag="v")

            onesap = ones[0, 0:1]
            ones_v = bass.AP(tensor=onesap.tensor, offset=onesap.offset,
                             ap=[[0, 128], [0, NKT * 4], [0, 1]])
            nc.gpsimd.dma_start(tv[:, :, :, 64:65], ones_v)

            # broadcast tri-mask [128,128] over the two head-halves
            def mask_b(n):
                m = tmsk[:]
                return bass.AP(tensor=m.tensor, offset=m.offset,
                               ap=[list(m.ap[0]), [0, 2], [1, n]])

            def proj_wave(scn):
                """One wave: q/k for s-chunk scn (4 units) + v for the 4
                s-tiles of chunk scn, spread over all PSUM pools so 8
                accumulations progress while xT chunks stream in.
                (PSUM accumulation groups are bank-granular, so one unit
                per bank.)"""
                sc = slice(scn * QC, (scn + 1) * QC)
                ptiles = [mmp.tile([128, 2 * QC], F32, tag="mm",
                                   name=f"pw{scn}_{i}") for i in range(2)]
                qk_units = []
                for r in range(2):
                    qk_units.append((ptiles[r][:, 0:QC], twq, r))
                    qk_units.append((ptiles[r][:, QC:2 * QC], twk, r))
                v_ps = [cxp.tile([128, QC], F32, tag="cx", name=f"pv{st}")
                        if i < 3 else
                        pyp.tile([128, QC], F32, tag="py", name=f"pv{st}")
                        for i, st in enumerate(range(4 * scn, 4 * scn + 4))]
                for e in range(NE):
                    for out_ap, w, r in qk_units:
                        nc.tensor.matmul(
                            out_ap, w[:, e, r * 128:(r + 1) * 128],
                            txt[e][:, sc],
                            start=(e == 0), stop=(e == NE - 1))
                    for i, st in enumerate(range(4 * scn, 4 * scn + 4)):
                        nc.tensor.matmul(
                            v_ps[i][:, 0:EC],
                            txt[e][:, st * 128:(st + 1) * 128], twv[:, e, :],
                            start=(e == 0), stop=(e == NE - 1))
                for r in range(2):
                    nc.vector.tensor_scalar_add(
                        tq[r][:, sc], ptiles[r][:, 0:QC], tbq[:, r:r + 1])
                    nc.vector.tensor_scalar_add(
                        tk[r][:, sc], ptiles[r][:, QC:2 * QC], tbk[:, r:r + 1])
                for i, st in enumerate(range(4 * scn, 4 * scn + 4)):
                    nc.vector.tensor_add(
                        tv[:, st, :, 0:64],
                        v_ps[i][:, 0:EC].rearrange("p (h d) -> p h d", h=4),
                        tbv[:].rearrange("p (h d) -> p h d", h=4))

            def attention(qc):
                n_kt = 4 * (qc + 1)
                ctx_sb = [None, None]
                for hp in range(2):
                    ctx_sbuf = ctxp.tile([128, QC], F32R, tag="ctxn",
                                         name=f"ctx{qc}_{hp}")
                    ctx_sb[hp] = ctx_sbuf
                    pctx = [cxp.tile([65, QC], F32, tag="cx",
                                     name=f"cx{qc}_{hp}_{i}")
                            for i in range(2)]
                    for kt in range(n_kt):
                        dg = kt - 4 * qc  # >=0: diagonal tile index
                        coff = 128 * dg if dg > 0 else 0
                        ps = mmp.tile([128, 2 * QC], F32, tag="mm",
                                      name=f"ps{qc}_{hp}_{kt}")
                        te = expp.tile([128, 2 * QC], F32R, tag="exp",
                                       name=f"te{qc}_{hp}_{kt}")
                        for h2 in range(2):
                            bp = h2 * 64
                            nc.tensor.matmul(
                                ps[:, h2 * QC + coff:(h2 + 1) * QC],
                                tk[hp][bp:bp + 64, kt * 128:(kt + 1) * 128],
                                tq[hp][bp:bp + 64,
                                       qc * QC + coff:(qc + 1) * QC],
                                start=True, stop=True)
                        if coff:
                            ps3 = ps[:].rearrange("p (t n) -> p t n", t=2)
                            te3 = te[:].rearrange("p (t n) -> p t n", t=2)
                            nc.scalar.activation(
                                te3[:, :, coff:QC], ps3[:, :, coff:QC], EXP)
                        else:
                            nc.scalar.activation(te[:], ps[:], EXP)
                        if dg >= 0:
                            te3 = te[:].rearrange("p (t n) -> p t n", t=2)
                            eng = nc.vector if cfg.get("mask_dve") else nc.gpsimd
                            eng.tensor_mul(
                                te3[:, :, coff:coff + 128],
                                te3[:, :, coff:coff + 128],
                                mask_b(128))
                        for h2 in range(2):
                            h = 2 * hp + h2
                            nc.tensor.matmul(
                                pctx[h2][:, coff:QC],
                                tv[:, kt, h, :],
                                te[:, h2 * QC + coff:(h2 + 1) * QC],
                                start=(kt == 0), stop=(kt == n_kt - 1))
                    # normalization (denominator = pctx row 64); odd head
                    # first so its partition-shift DMA overlaps the even mul
                    for h2 in (1, 0):
                        rec1 = rows.tile([1, QC], F32, tag="rec1")
                        nc.vector.reciprocal(rec1[:], pctx[h2][64:65, :])
                        rec = rows.tile([64, QC], F32, tag="rec")
                        nc.gpsimd.partition_broadcast(rec[:], rec1[:])
                        if h2 == 0:
                            nc.vector.tensor_mul(
                                ctx_sbuf[0:64, :], pctx[h2][0:64, :], rec[:])
                        else:
                            tmp = oddp.tile([64, QC], F32R, tag="odd")
                            nc.vector.tensor_mul(
                                tmp[:], pctx[h2][0:64, :], rec[:])
                            nc.sync.dma_start(ctx_sbuf[64:128, :], tmp[:])
                return ctx_sb

            def out_proj(qc, ctx_sb, last=False):
                for ss in range(4):
                    s0 = qc * QC + ss * 128
                    for nn in range(2):
                        # the final q-chunk's projections also draw from the
                        # (idle by then) scores pool for deeper pipelining
                        if last and (ss * 2 + nn) % 2 == 1:
                            py = mmp.tile([128, 2 * QC], F32, tag="mm",
                                          name=f"py{qc}_{ss}_{nn}")
                        else:
                            py = pyp.tile([128, QC], F32, tag="py",
                                          name=f"py{qc}_{ss}_{nn}")
                        for hp in range(2):
                            nc.tensor.matmul(
                                py[:, 0:QC],
                                ctx_sb[hp][:, ss * 128:(ss + 1) * 128],
                                two[:, hp, nn * QC:(nn + 1) * QC],
                                start=(hp == 0), stop=(hp == 1))
                        ysb = yp.tile([128, QC], F32, tag="y",
                                      name=f"y{qc}_{ss}_{nn}")
                        if cfg.get("ycopy", "dve") == "act":
                            nc.scalar.copy(ysb[:], py[:, 0:QC])
                        else:
                            nc.vector.tensor_copy(ysb[:], py[:, 0:QC])
                        nc.sync.dma_start(
                            y[s0:s0 + 128, nn * QC:(nn + 1) * QC], ysb[:])

            # interleave projection blocks with attention q-chunks; process
            # the longest q-chunk right after projections and end on the
            # shortest to minimize the kernel tail
            if cfg.get("inner"):
                for blk in range(NQC):
                    proj_wave(blk)
                    if blk >= 1:
                        out_proj(blk - 1, attention(blk - 1))
                out_proj(NQC - 1, attention(NQC - 1), last=True)
            else:
                qc_order = cfg.get("qc_order", [0, 1, 2, 3])
                for blk in range(NQC):
                    proj_wave(blk)
                    if blk == 1:
                        out_proj(0, attention(0))
                for qc in qc_order[1:]:
                    out_proj(qc, attention(qc), last=(qc == qc_order[-1]))

    nc.compile()
    return nc


def _get_nc():
    if "nc" not in _CACHE:
        _CACHE["nc"] = _build_nc()
    return _CACHE["nc"]


def make_mask():
    kl = np.arange(128)[:, None]
    ql = np.arange(128)[None, :]
    return (ql >= kl).astype(np.float32)


def shard_inputs(x, Wq, bq, Wk, bk, Wv, bv, Wo, bo):
    """Build the 8 per-core input maps (host-side sharding)."""
    x = np.asarray(x, dtype=np.float32)
    scale = np.float32(1.0 / np.sqrt(D))
    mask = make_mask()
    ones = np.ones((1, 64), np.float32)
    in_maps = []
    xTb = [np.ascontiguousarray(np.asarray(x[b]).T) for b in range(B)]
    for c in range(NCORES):
        b, g = divmod(c, 4)
        cs = slice(g * EC, (g + 1) * EC)
        in_maps.append({
            "xT": xTb[b],
            "wq": np.ascontiguousarray(np.asarray(Wq[:, cs]) * scale),
            "wk": np.ascontiguousarray(np.asarray(Wk[:, cs])),
            "wv": np.ascontiguousarray(np.asarray(Wv[:, cs])),
            "wo": np.ascontiguousarray(np.asarray(Wo[cs, :])),
            "bq": (np.asarray(bq[cs]) * scale).reshape(2, 128, 1).astype(np.float32),
            "bk": np.asarray(bk[cs]).reshape(2, 128, 1).astype(np.float32),
            "bv": np.asarray(bv[cs]).reshape(1, EC).astype(np.float32),
            "msk": mask,
            "ones": ones,
        })
    return in_maps


def combine_outputs(results, bo):
    y = np.zeros((B, S, E), np.float32)
    for c in range(NCORES):
        b = c // 4
        y[b] += results[c]["y"]
    y += np.asarray(bo, dtype=np.float32)[None, None, :]
    return y


def kernel(x, Wq, bq, Wk, bk, Wv, bv, Wo, bo):
    from concourse.bass_utils import run_bass_kernel_spmd

    nc = _get_nc()
    in_maps = shard_inputs(x, Wq, bq, Wk, bk, Wv, bv, Wo, bo)
    try:
        res = run_bass_kernel_spmd(nc, in_maps, core_ids=list(range(NCORES)))
    except Exception:
        # transient device errors (e.g. a wedged core) usually clear on retry
        res = run_bass_kernel_spmd(nc, in_maps, core_ids=list(range(NCORES)))
    return combine_outputs(res.results, bo)



# revision 47
# speedup vs baseline: 1.7283x; 1.7283x over previous
"""Causal self-attention (B=2, S=2048, E=1024, H=16) on 8 TRN2 NeuronCores.

Sharding: core c = 4*b + g handles batch b and head-group g (4 heads,
256 E-columns). Each core computes q/k/v projections for its head slice,
causal attention for its 4 heads, and a partial output projection
y_c = ctx_g @ Wo[rows_g]. Host sums the 4 partials per batch and adds bo.

Engine plan (CoreSim cost model driven):
  PE   - QKV projections in fp8e4 DoubleRow (K=256/instr, 0.5 cyc/row) with
         3-term error compensation (x8@W8 + x8@rW8 + rx8@W8) accumulated in
         one PSUM group; scores kT.T@q in bf16 (1 cyc/row, no >=256-free
         constraint so causal trimming is exact); A@V in natural orientation
         (out = [128 q-parts, 65], bf16, denominator rides as V's ones
         column); output projection in bf16 from a folded ctxT.
  ACT  - exp only (the hard floor: ~8.4M exps/core).
  DVE  - q/k bias+descale (psum->bf16), reciprocal of denominators,
         ctx normalize (psum * recip-broadcast -> bf16).
  Pool - v bias+descale, causal mask multiply on te, y psum->sbuf copies.
  DMA  - fp8/bf16 inputs (~10x fewer bytes than f32), XBAR transpose of the
         normalized ctx ([128,128] bf16 blocks) to build ctxT off-engine.

PSUM banks (8): scores/qk-proj ring [128,2,512]x2 = 4, ctx/v-proj ring
[128,2,512]x1 = 2 (head-per-bank), out-proj [128,512]x2 = 2.
"""

import os

import numpy as np
import ml_dtypes

os.environ.setdefault("NEURON_RT_RESET_CORES", "1")

B, S, E, H, D = 2, 2048, 1024, 16, 64
NCORES = 8
EC = 256          # E-columns per core (4 heads x 64)
QC = 512          # q-chunk width
NQC = S // QC     # 4
NKT = S // 128    # 16 k-tiles
NKS = 4           # fp8 DoubleRow K-steps (4 x 256 = 1024)

WSQ = 256.0       # fp8 range scale for Wq*(1/8)
WSK = 64.0        # fp8 range scale for Wk / Wv
E4NP = ml_dtypes.float8_e4m3
BFNP = ml_dtypes.bfloat16

_CACHE = {}


def _build_nc(cfg=None):
    cfg = cfg or {}
    import concourse.mybir as mybir
    import concourse.tile as tile
    import concourse.bass as bass
    from concourse import bacc

    F32 = mybir.dt.float32
    BF16 = mybir.dt.bfloat16
    F8 = mybir.dt.float8e4
    EXP = mybir.ActivationFunctionType.Exp
    DR = mybir.MatmulPerfMode.DoubleRow
    MUL = mybir.AluOpType.mult
    ADD = mybir.AluOpType.add

    TE_BUFS = cfg.get("te", 44)
    nc = bacc.Bacc("TRN2", target_bir_lowering=False, debug=False)

    x8 = nc.dram_tensor("x8", [128, NKS, 2, S], F8, kind="ExternalInput")
    rx8 = nc.dram_tensor("rx8", [128, NKS, 2, S], F8, kind="ExternalInput")
    wq8 = nc.dram_tensor("wq8", [128, NKS, 2, EC], F8, kind="ExternalInput")
    wk8 = nc.dram_tensor("wk8", [128, NKS, 2, EC], F8, kind="ExternalInput")
    wv8 = nc.dram_tensor("wv8", [128, NKS, 2, EC], F8, kind="ExternalInput")
    rwq8 = nc.dram_tensor("rwq8", [128, NKS, 2, EC], F8, kind="ExternalInput")
    rwk8 = nc.dram_tensor("rwk8", [128, NKS, 2, EC], F8, kind="ExternalInput")
    rwv8 = nc.dram_tensor("rwv8", [128, NKS, 2, EC], F8, kind="ExternalInput")
    wo = nc.dram_tensor("wo", [128, 2, E], BF16, kind="ExternalInput")
    bq = nc.dram_tensor("bq", [2, 128, 1], F32, kind="ExternalInput")
    bk = nc.dram_tensor("bk", [2, 128, 1], F32, kind="ExternalInput")
    bv = nc.dram_tensor("bv", [1, EC], F32, kind="ExternalInput")
    msk = nc.dram_tensor("msk", [128, 128], BF16, kind="ExternalInput")
    ones = nc.dram_tensor("ones", [1, 64], BF16, kind="ExternalInput")

    y = nc.dram_tensor("y", [S, E], BF16, kind="ExternalOutput")

    with tile.TileContext(nc) as tc:
        with (
            tc.tile_pool(name="weights", bufs=1) as wpool,
            tc.tile_pool(name="xp", bufs=1) as xp,
            tc.tile_pool(name="qkv", bufs=1) as qkv,
            tc.tile_pool(name="tep", bufs=TE_BUFS) as tep,
            tc.tile_pool(name="tcnp", bufs=4) as tcnp,
            tc.tile_pool(name="rcp", bufs=4) as rcp,
            tc.tile_pool(name="ctp", bufs=1) as ctp,
            tc.tile_pool(name="typ", bufs=6) as typ,
            tc.tile_pool(name="smalls", bufs=1) as smalls,
            tc.tile_pool(name="scp", bufs=2, space="PSUM") as scp,
            tc.tile_pool(name="cxp", bufs=2, space="PSUM") as cxp,
            tc.tile_pool(name="prp", bufs=2, space="PSUM") as prp,
        ):
            # ---- small constants (Pool SWDGE queue) ----
            tbq = smalls.tile([128, 2], F32, tag="bq")
            tbk = smalls.tile([128, 2], F32, tag="bk")
            tbv = smalls.tile([128, EC], F32, tag="bv")
            tmsk = smalls.tile([128, 128], BF16, tag="msk")

            for r in range(2):
                nc.gpsimd.dma_start(tbq[:, r:r + 1], bq[r])
                nc.gpsimd.dma_start(tbk[:, r:r + 1], bk[r])
            bvap = bv[0, :]
            bv_b = bass.AP(tensor=bvap.tensor, offset=bvap.offset,
                           ap=[[0, 128]] + list(bvap.ap))
            nc.gpsimd.dma_start(tbv[:], bv_b)
            nc.gpsimd.dma_start(tmsk[:], msk[:])

            # ---- x fp8 (+ residual), loaded per 512-wide wave ----
            tx = xp.tile([128, NKS, 2, S], F8, tag="x8")
            trx = xp.tile([128, NKS, 2, S], F8, tag="rx8")

            def ldx(w):
                sl = slice(w * QC, (w + 1) * QC)
                nc.scalar.dma_start(tx[:, :, :, sl], x8[:, :, :, sl])
                nc.scalar.dma_start(trx[:, :, :, sl], rx8[:, :, :, sl])

            # wave-0 x + q/k weights first so the q/k chain starts ASAP
            sl0 = slice(0, QC)
            tw = {}

            def ldw(nm, dram):
                t = wpool.tile([128, NKS, 2, EC], F8, tag=nm, name=nm)
                nc.sync.dma_start(t[:], dram[:])
                tw[nm] = t

            for ks in range(NKS):
                nc.scalar.dma_start(tx[:, ks, :, sl0], x8[:, ks, :, sl0])
            ldw("wq", wq8)
            ldw("wk", wk8)
            ldw("rwq", rwq8)
            ldw("rwk", rwk8)
            nc.scalar.dma_start(trx[:, :, :, sl0], rx8[:, :, :, sl0])
            ldw("wv", wv8)
            ldw("rwv", rwv8)
            two = wpool.tile([128, 2, E], BF16, tag="wo")
            nc.sync.dma_start(two[:], wo[:])
            for w_ in range(1, NQC):
                ldx(w_)

            # ---- persistent activations ----
            SCF8 = cfg.get("sc_fp8", True)
            QKDT = F8 if SCF8 else BF16
            # natural layout (partition = feature within r-tile)
            tq = [qkv.tile([128, S], QKDT, tag=f"q{r}", name=f"q{r}")
                  for r in range(2)]
            tk = [qkv.tile([128, S], QKDT, tag=f"k{r}", name=f"k{r}")
                  for r in range(2)]
            # folded fp8 layout: partition = head_local*32 + d_low,
            # slot = d_high half; built by partition-shift DMAs
            if SCF8:
                tqf = qkv.tile([128, 2, S], F8, tag="qf", name="qf")
                tkf = qkv.tile([128, 2, S], F8, tag="kf", name="kf")
            # v1: [128, kt, head, 65]; col 64 of each head block = 1.0
            tv = qkv.tile([128, NKT, 4, 65], BF16, tag="v")
            onesap = ones[0, 0:1]
            ones_v = bass.AP(tensor=onesap.tensor, offset=onesap.offset,
                             ap=[[0, 128], [0, NKT * 4], [0, 1]])
            nc.gpsimd.dma_start(tv[:, :, :, 64:65], ones_v)

            tct = ctp.tile([128, 2, S], BF16, tag="ct")  # folded ctxT

            def mask_b(n):
                m = tmsk[:]
                return bass.AP(tensor=m.tensor, offset=m.offset,
                               ap=[list(m.ap[0]), [0, 2], [1, n]])

            def colb(ap1, n):
                # [128, 1] AP -> [128, n] stride-0 broadcast
                return bass.AP(tensor=ap1.tensor, offset=ap1.offset,
                               ap=[list(ap1.ap[0]), [0, n]])

            def colb2(ap2, n):
                # [128, k] AP -> [128, k, n] stride-0 broadcast
                return bass.AP(tensor=ap2.tensor, offset=ap2.offset,
                               ap=[list(ap2.ap[0]), list(ap2.ap[1]), [0, n]])

            def apx(t, off, dims):
                # partition dim of tile t + custom free dims at f32-col offset
                a = t[:]
                return bass.AP(tensor=a.tensor, offset=a.offset + off,
                               ap=[list(a.ap[0])] + [list(d) for d in dims])

            def order(later, first):
                bass._add_dep_helper(later.ins, first.ins, sync=False,
                                     reason="psum zero-region order")

            COMP = ((None, tx), ("r", tx), (None, trx))  # (w-residual?, x-tensor)

            def qk_unit(w, r, wn):
                dst, ws, bias = ((tq, WSQ, tbq) if wn == "wq"
                                 else (tk, WSK, tbk))
                sl = slice(w * QC, (w + 1) * QC)
                prt = prp.tile([128, QC], F32, tag="pr", name=f"p{wn}{w}_{r}")
                idx = 0
                for res, xt in COMP:
                    wt = tw[("r" if res else "") + wn]
                    for ks in range(NKS):
                        nc.tensor.matmul(
                            prt[:],
                            wt[:, ks, :, r * 128:(r + 1) * 128],
                            xt[:, ks, :, sl],
                            start=(idx == 0), stop=(idx == 3 * NKS - 1),
                            perf_mode=DR)
                        idx += 1
                nc.vector.scalar_tensor_tensor(
                    dst[r][:, sl], prt[:], 1.0 / ws,
                    colb(bias[:, r:r + 1], QC), op0=MUL, op1=ADD)
                if SCF8:
                    fdst = tqf if wn == "wq" else tkf
                    for h2 in range(2):
                        for slot in range(2):
                            nc.sync.dma_start(
                                fdst[(2 * r + h2) * 32:(2 * r + h2) * 32 + 32,
                                     slot, sl],
                                dst[r][h2 * 64 + slot * 32:
                                       h2 * 64 + slot * 32 + 32, sl])

            def v_unit(w, pair):
                cxt = cxp.tile([128, 512], F32, tag="cx", name=f"pv{w}_{pair}")
                first = None
                for st2 in range(2):
                    st = 4 * w + 2 * pair + st2
                    idx = 0
                    for res, xt in COMP:
                        wt = tw[("r" if res else "") + "wv"]
                        for ks in range(NKS):
                            m = nc.tensor.matmul(
                                cxt[:, st2 * EC:(st2 + 1) * EC],
                                xt[:, ks, :, st * 128:(st + 1) * 128],
                                wt[:, ks, :, :],
                                start=(first is None), stop=False,
                                perf_mode=DR, skip_group_check=True)
                            if first is None:
                                first = m
                            elif idx == 0:
                                order(m, first)
                            idx += 1
                for st2 in range(2):
                    st = 4 * w + 2 * pair + st2
                    nc.vector.scalar_tensor_tensor(
                        tv[:, st, :, 0:64],
                        apx(cxt, st2 * EC, [[64, 4], [1, 64]]),
                        1.0 / WSK,
                        tbv[:].rearrange("p (h d) -> p h d", h=4),
                        op0=MUL, op1=ADD)

            def proj_wave(w):
                for r in range(2):
                    qk_unit(w, r, "wq")
                    qk_unit(w, r, "wk")
                for p in range(2):
                    v_unit(w, p)

            all_tes = {}

            def scores_phase(qc):
                n_kt = 4 * (qc + 1)
                tes = [[], []]
                all_tes[qc] = tes
                for hp in range(2):
                    for kt in range(n_kt):
                        dg = kt - 4 * qc
                        coff = 128 * dg if dg > 0 else 0
                        sct = scp.tile([128, 2, QC], F32, tag="sc",
                                       name=f"s{qc}_{hp}_{kt}")
                        for h2 in range(2):
                            if SCF8:
                                hl = (2 * hp + h2) * 32
                                nc.tensor.matmul(
                                    sct[:, h2, coff:QC],
                                    tkf[hl:hl + 32, :,
                                        kt * 128:(kt + 1) * 128],
                                    tqf[hl:hl + 32, :,
                                        qc * QC + coff:(qc + 1) * QC],
                                    start=True, stop=True, perf_mode=DR,
                                    tile_position=(hl, 0))
                            else:
                                bp = h2 * 64
                                nc.tensor.matmul(
                                    sct[:, h2, coff:QC],
                                    tk[hp][bp:bp + 64, kt * 128:(kt + 1) * 128],
                                    tq[hp][bp:bp + 64,
                                           qc * QC + coff:(qc + 1) * QC],
                                    start=True, stop=True)
                        tet = tep.tile([128, 2, QC], BF16, tag="te",
                                       name=f"te{qc}_{hp}_{kt}")
                        nc.scalar.activation(
                            tet[:, :, coff:QC], sct[:, :, coff:QC], EXP)
                        if dg >= 0:
                            nc.gpsimd.tensor_mul(
                                tet[:, :, coff:coff + 128],
                                tet[:, :, coff:coff + 128],
                                mask_b(128))
                        tes[hp].append(tet)

            def ctx_phase(qc, op_inline=False):
                tes = all_tes[qc]
                for sti in range(4):
                    st = 4 * qc + sti
                    cxt = cxp.tile([128, 512], F32, tag="cx",
                                   name=f"cx{qc}_{sti}")
                    first = None
                    for hp in range(2):
                        for h2 in range(2):
                            h = 2 * hp + h2
                            for kt in range(st + 1):
                                m = nc.tensor.matmul(
                                    cxt[:, h * 128:h * 128 + 65],
                                    tes[hp][kt][:, h2,
                                                sti * 128:(sti + 1) * 128],
                                    tv[:, kt, h, :],
                                    start=(first is None), stop=False,
                                    skip_group_check=True)
                                if first is None:
                                    first = m
                                elif kt == 0:
                                    order(m, first)
                    rc = rcp.tile([128, 4], F32, tag="rc",
                                  name=f"rc{qc}_{sti}")
                    nc.vector.reciprocal(rc[:], apx(cxt, 64, [[128, 4], [1, 1]]))
                    tcn = tcnp.tile([128, 4, 64], BF16, tag="cn",
                                    name=f"cn{qc}_{sti}")
                    nc.vector.tensor_mul(
                        tcn[:], apx(cxt, 0, [[128, 4], [1, 64]]),
                        colb2(rc[:], 64))
                    for hp in range(2):
                        nc.sync.dma_start_transpose(
                            tct[:, hp, st * 128:(st + 1) * 128],
                            tcn[:, 2 * hp:2 * hp + 2, :])
                    if op_inline:
                        out_proj_sti(qc, sti)

            def out_proj_sti(qc, sti):
                    st = 4 * qc + sti
                    for nn in range(2):
                        pyt = prp.tile([128, QC], F32, tag="pr",
                                       name=f"py{qc}_{sti}_{nn}")
                        for hp in range(2):
                            nc.tensor.matmul(
                                pyt[:],
                                tct[:, hp, st * 128:(st + 1) * 128],
                                two[:, hp, nn * QC:(nn + 1) * QC],
                                start=(hp == 0), stop=(hp == 1))
                        ty = typ.tile([128, QC], BF16, tag="y",
                                      name=f"y{qc}_{sti}_{nn}")
                        nc.vector.tensor_copy(ty[:], pyt[:])
                        nc.sync.dma_start(
                            y[st * 128:(st + 1) * 128,
                              nn * QC:(nn + 1) * QC], ty[:])

            def out_proj(qc):
                for sti in range(4):
                    out_proj_sti(qc, sti)

            # Emission = scheduler priority. The exp stream is the global
            # pacer: scores/exp chains first (chunk order) with the next
            # wave's q/k projection between them, then all deferrable work.
            CORDER = cfg.get("corder", [0, 1, 3, 2])
            # waves needed before a chunk's scores: all w <= qc
            emitted_qk = set()

            def need_qk(qc):
                for w_ in range(qc + 1):
                    if w_ not in emitted_qk:
                        emitted_qk.add(w_)
                        for r in range(2):
                            qk_unit(w_, r, "wq")
                            qk_unit(w_, r, "wk")

            need_qk(CORDER[0])
            for i, qc in enumerate(CORDER):
                scores_phase(qc)
                if i + 1 < NQC:
                    need_qk(CORDER[i + 1])
            emitted_v = set()

            def need_v(qc):
                for w_ in range(qc + 1):
                    if w_ not in emitted_v:
                        emitted_v.add(w_)
                        for p in range(2):
                            v_unit(w_, p)

            need_v(CORDER[0])
            for i, qc in enumerate(CORDER):
                if i + 1 < NQC:
                    need_v(CORDER[i + 1])
                ctx_phase(qc)
                out_proj(qc)

    nc.compile()
    return nc


def _get_nc():
    if "nc" not in _CACHE:
        _CACHE["nc"] = _build_nc()
    return _CACHE["nc"]


def make_mask():
    kl = np.arange(128)[:, None]
    ql = np.arange(128)[None, :]
    return (ql >= kl).astype(BFNP)


def _fold(t):
    # [E, N] -> [128, NKS, 2, N] with e = ks*256 + sl*128 + p
    n = t.shape[1]
    return np.ascontiguousarray(
        t.reshape(NKS, 2, 128, n).transpose(2, 0, 1, 3))


def _q8(t):
    t8 = t.astype(E4NP)
    return t8, (t - t8.astype(np.float32)).astype(E4NP)


def shard_inputs(x, Wq, bq, Wk, bk, Wv, bv, Wo, bo):
    x = np.asarray(x, dtype=np.float32)
    scale = np.float32(1.0 / np.sqrt(D))
    mask = make_mask()
    ones = np.ones((1, 64), BFNP)
    in_maps = []
    xf = []
    for b in range(B):
        x8, rx8 = _q8(_fold(np.ascontiguousarray(x[b].T)))
        xf.append((x8, rx8))
    for c in range(NCORES):
        b, g = divmod(c, 4)
        cs = slice(g * EC, (g + 1) * EC)
        wq, rwq = _q8(_fold(np.asarray(Wq[:, cs]) * (scale * WSQ)))
        wk, rwk = _q8(_fold(np.asarray(Wk[:, cs]) * WSK))
        wv, rwv = _q8(_fold(np.asarray(Wv[:, cs]) * WSK))
        in_maps.append({
            "x8": xf[b][0], "rx8": xf[b][1],
            "wq8": wq, "rwq8": rwq,
            "wk8": wk, "rwk8": rwk,
            "wv8": wv, "rwv8": rwv,
            "wo": np.ascontiguousarray(
                np.asarray(Wo[cs, :]).reshape(2, 128, E).transpose(1, 0, 2)
            ).astype(BFNP),
            "bq": (np.asarray(bq[cs]) * scale).reshape(2, 128, 1).astype(np.float32),
            "bk": np.asarray(bk[cs]).reshape(2, 128, 1).astype(np.float32),
            "bv": np.asarray(bv[cs]).reshape(1, EC).astype(np.float32),
            "msk": mask,
            "ones": ones,
        })
    return in_maps


def combine_outputs(results, bo):
    y = np.zeros((B, S, E), np.float32)
    for c in range(NCORES):
        b = c // 4
        y[b] += np.asarray(results[c]["y"], dtype=np.float32)
    y += np.asarray(bo, dtype=np.float32)[None, None, :]
    return y


def kernel(x, Wq, bq, Wk, bk, Wv, bv, Wo, bo):
    from concourse.bass_utils import run_bass_kernel_spmd

    nc = _get_nc()
    in_maps = shard_inputs(x, Wq, bq, Wk, bk, Wv, bv, Wo, bo)
    try:
        res = run_bass_kernel_spmd(nc, in_maps, core_ids=list(range(NCORES)))
    except Exception:
        # transient device errors (e.g. a wedged core) usually clear on retry
        res = run_bass_kernel_spmd(nc, in_maps, core_ids=list(range(NCORES)))
    return combine_outputs(res.results, bo)
